# revision 24
# baseline (speedup 1.0000x reference)
import os
import sys

sys.path.insert(0, "/opt/trn_rl_repo")

import numpy as np
import ml_dtypes
import bass_rust
from concourse import bass, mybir
from concourse.tile import TileContext
from concourse.vector_clock import ScopedClock
from concourse.bass_utils import run_bass_kernel_spmd

B, S, E, H = 4, 2048, 1024, 1024
NCORES = 8
NT = 8  # q-tiles per core, 128 rows each
SH = S // 2  # key half per core
EC = E // 128  # contraction chunks
HC = H // 128  # h chunks
F32 = mybir.dt.float32
BF16 = mybir.dt.bfloat16
BF = ml_dtypes.bfloat16

# Results of the last run_bass_kernel_spmd call (for test harness inspection).
LAST_RESULT = None


def _global_tile(core: int, t: int) -> int:
    """Local q-tile t on core -> global 128-row tile index r in [0, 16).

    Tiles are grouped in four causal classes (nkc = 4*(t//2+1) key-chunks of
    128); each core takes two tiles per class so instruction streams are
    identical across cores.
    """
    half = core % 2
    return 4 * (t // 2) + 2 * half + (t % 2)


class PatchedTileContext(TileContext):
    """TileContext whose tail drain carries at most one sem wait.

    The walrus codegen in this container rejects a Drain with more than one
    sync wait ("Too many sync wait commands"); split the global-clock waits
    across a chain of drains on the same engine instead.
    """

    def _drain_and_barrier(self, tick_clock, wait_clock):
        drain_inst = self.nc.sync.drain()
        wait_clock.add_sem_waits(
            drain_inst.ins, ScopedClock({None: tick_clock.global_clock})
        )
        mi = drain_inst.ins
        waits = list(mi.sync_info.on_wait)
        ups = list(mi.sync_info.on_update)
        if len(waits) > 1:
            mi.sync_info = bass_rust.SyncInfo(on_wait=waits[:1], on_update=[])
            for i, w in enumerate(waits[1:]):
                d2 = self.nc.sync.drain()
                last = i == len(waits) - 2
                d2.ins.sync_info = bass_rust.SyncInfo(
                    on_wait=[w], on_update=ups if last else []
                )
        self.nc.all_engine_barrier()
        assert self.sems is not None
        popped = self.nc._tile_sem_poison_stack.pop()
        assert popped is self._sem_poison
        self.nc.clear_and_free_semaphores(list(self.sems.allocated().values()))
        self.nc.all_engine_barrier()


def _split_multi_waits(json_bytes):
    """Rewrite BIR so no instruction carries more than one sync wait.

    The walrus build in this container rejects instructions with multiple
    sync waits ("Too many sync wait commands"). Engines execute in order, so
    hoisting the extra waits onto NoOp instructions inserted immediately
    before the original instruction is semantically equivalent.
    """
    import json as _json

    d = _json.loads(json_bytes)
    ctr = 0
    for f in d.get("functions", []):
        for blk in f.get("blocks", []):
            insts = blk.get("instructions", [])
            out = []
            for inst in insts:
                si = inst.get("sync_info") or {}
                ow = si.get("on_wait") or []
                if len(ow) > 1:
                    for w in ow[:-1]:
                        out.append(
                            {
                                "debug": inst.get("debug", 0),
                                "engine": inst["engine"],
                                "ins": [],
                                "name": f"wsplit_{ctr}",
                                "opcode": "NoOp",
                                "outs": [],
                                "sync_info": {"on_update": [], "on_wait": [w]},
                            }
                        )
                        ctr += 1
                    si = dict(si)
                    si["on_wait"] = [ow[-1]]
                    inst = dict(inst)
                    inst["sync_info"] = si
                out.append(inst)
            blk["instructions"] = out
    return _json.dumps(d).encode()


def _build_program():
    nc = bass.Bass("TRN2", target_bir_lowering=False, debug=False, num_devices=NCORES)
    orig_to_json_bytes = nc.to_json_bytes
    nc.to_json_bytes = lambda: _split_multi_waits(orig_to_json_bytes())

    xh = nc.dram_tensor("xh", [E, SH], BF16, kind="ExternalInput")
    xq = nc.dram_tensor("xq", [E, NT * 128], BF16, kind="ExternalInput")
    wqT = nc.dram_tensor("wqT", [E, H], BF16, kind="ExternalInput")
    wkT = nc.dram_tensor("wkT", [E, H], BF16, kind="ExternalInput")
    wvT = nc.dram_tensor("wvT", [E, H], BF16, kind="ExternalInput")
    bqs = nc.dram_tensor("bqs", [H], F32, kind="ExternalInput")
    bk = nc.dram_tensor("bk", [H], F32, kind="ExternalInput")
    bv = nc.dram_tensor("bv", [H], F32, kind="ExternalInput")
    masks = nc.dram_tensor("masks", [4, 128, 4, 256], BF16, kind="ExternalInput")
    out = nc.dram_tensor("out", [NT, 128, H], F32, kind="ExternalOutput")

    GROUPS = [[0, 1], [2, 3], [4, 5], [6, 7]]

    with PatchedTileContext(nc) as tc:
        with (
            tc.tile_pool(name="const", bufs=1) as const_pool,
            tc.tile_pool(name="ktp", bufs=1) as kt_pool,
            tc.tile_pool(name="vp", bufs=1) as v_pool,
            tc.tile_pool(name="qtp", bufs=1) as qt_pool,
            tc.tile_pool(name="dram", bufs=1, space="DRAM") as dram_pool,
        ):
            cst = const_pool.tile([128, H + 2 * HC], F32, tag="cst")
            bv_bc = cst[:, 0:H]
            bq_t = cst[:, H : H + HC]
            bk_t = cst[:, H + HC : H + 2 * HC]
            ones = const_pool.tile([128, 1], BF16, tag="ones")
            nc.gpsimd.dma_start(out=bq_t, in_=bqs[:].rearrange("(c p) -> p c", p=128))
            nc.gpsimd.dma_start(out=bk_t, in_=bk[:].rearrange("(c p) -> p c", p=128))
            nc.gpsimd.dma_start(out=bv_bc, in_=bv[:].partition_broadcast(128))
            nc.gpsimd.memset(ones, 1.0)

            kt = [
                kt_pool.tile([128, S], BF16, tag=f"kt{c}", name=f"kt{c}")
                for c in range(HC)
            ]
            v_sb = [
                v_pool.tile([128, H], BF16, tag=f"v{k}", name=f"v{k}")
                for k in range(16)
            ]
            qt_all = qt_pool.tile([128, 4, HC, 256], BF16, tag="qt")

            kb_in = [
                dram_pool.tile([HC, 128, 512], BF16, tag=f"kbi{s}", name=f"kbi{s}")
                for s in range(2)
            ]
            kb_out = [
                dram_pool.tile([2, HC, 128, 512], BF16, tag=f"kbo{s}", name=f"kbo{s}")
                for s in range(2)
            ]
            vb_in = dram_pool.tile([8, 128, H], BF16, tag="vbi")
            vb_out = dram_pool.tile([2, 8, 128, H], BF16, tag="vbo")
            warm_in = dram_pool.tile([128, 1], F32, tag="wmi")
            warm_out = dram_pool.tile([2, 128, 1], F32, tag="wmo")
            # tiny dummy collective fired immediately: absorbs the CC
            # engine's one-time startup latency before the real gathers
            warm_sb = const_pool.tile([128, 1], F32, tag="wms")
            nc.gpsimd.memset(warm_sb, 0.0)
            nc.gpsimd.dma_start(out=warm_in[:, :], in_=warm_sb)
            nc.gpsimd.collective_compute(
                "AllGather",
                mybir.AluOpType.bypass,
                replica_groups=GROUPS,
                ins=[warm_in[:, :].opt()],
                outs=[warm_out[:, :, :].opt()],
            )

            with (
                tc.tile_pool(name="xhp", bufs=1) as xh_pool,
                tc.tile_pool(name="wA", bufs=1) as wA_pool,
                tc.tile_pool(name="wB", bufs=1) as wB_pool,
                tc.tile_pool(name="xqp", bufs=1) as xq_pool,
                tc.tile_pool(name="stg", bufs=2) as stg_pool,
                tc.tile_pool(name="pps", bufs=8, space="PSUM") as pps_pool,
            ):
                xh_sb = xh_pool.tile([128, EC, SH], BF16, tag="xh")
                wk_sb = [
                    wA_pool.tile([128, H], BF16, tag=f"wA{e}", name=f"wk{e}")
                    for e in range(EC)
                ]
                wv_sb = [
                    wB_pool.tile([128, H], BF16, tag=f"wB{e}", name=f"wv{e}")
                    for e in range(EC)
                ]
                # interleave x chunks with wk chunks so the e=0 matmul can
                # issue after the first two transfers; wv prefetches behind.
                # e=0 loads only the first 512 key-columns up front so the
                # first matmul's data lands as early as possible.
                nc.sync.dma_start(out=wk_sb[0], in_=wkT[0:128, :])
                nc.sync.dma_start(out=xh_sb[:, 0, 0:512], in_=xh[0:128, 0:512])
                for e in range(1, EC):
                    nc.sync.dma_start(
                        out=wk_sb[e], in_=wkT[e * 128 : (e + 1) * 128, :]
                    )
                    nc.sync.dma_start(
                        out=xh_sb[:, e, 0:512], in_=xh[e * 128 : (e + 1) * 128, 0:512]
                    )
                for e in range(EC):
                    nc.sync.dma_start(
                        out=xh_sb[:, e, 512:SH], in_=xh[e * 128 : (e + 1) * 128, 512:SH]
                    )
                for e in range(EC):
                    nc.sync.dma_start(out=wv_sb[e], in_=wvT[e * 128 : (e + 1) * 128, :])

                # ---- phases K/V interleaved: each 512-key piece stages and
                # kicks its pairwise AllGather as early as possible so the
                # CC chain (K0, V0, K1, V1) hides under later projections.
                kth = stg_pool.tile([128, HC, SH], BF16, tag="kth", bufs=1)

                def k_slice(sl):
                    psk = [
                        pps_pool.tile([128, 512], F32, tag="pps", name=f"psk{hc}")
                        for hc in range(HC)
                    ]
                    for e in range(EC):
                        for hc in range(HC):
                            nc.tensor.matmul(
                                psk[hc],
                                lhsT=wk_sb[e][:, hc * 128 : (hc + 1) * 128],
                                rhs=xh_sb[:, e, sl * 512 : (sl + 1) * 512],
                                start=(e == 0),
                                stop=(e == EC - 1),
                            )
                    for hc in range(HC):
                        nc.vector.tensor_scalar_add(
                            kth[:, hc, sl * 512 : (sl + 1) * 512],
                            psk[hc],
                            bk_t[:, hc : hc + 1],
                        )
                        nc.sync.dma_start(
                            out=kb_in[sl][hc, :, :],
                            in_=kth[:, hc, sl * 512 : (sl + 1) * 512],
                        )
                    nc.gpsimd.collective_compute(
                        "AllGather",
                        mybir.AluOpType.bypass,
                        replica_groups=GROUPS,
                        ins=[kb_in[sl][:, :, :].opt()],
                        outs=[kb_out[sl][:, :, :, :].opt()],
                    )
                    # readbacks trigger from scalar so they don't head-block
                    # the sync queue on the CC-done semaphore
                    for r in range(2):
                        for hc in range(HC):
                            nc.scalar.dma_start(
                                out=kt[hc][
                                    :, r * SH + sl * 512 : r * SH + (sl + 1) * 512
                                ],
                                in_=kb_out[sl][r, hc, :, :],
                            )

                def v_group(g):
                    psv = [
                        [
                            pps_pool.tile(
                                [128, 512], F32, tag="pps", name=f"psv{j}_{hh}"
                            )
                            for hh in range(2)
                        ]
                        for j in range(4)
                    ]
                    for e in range(EC):
                        for j in range(4):
                            kc = g * 4 + j
                            for hh in range(2):
                                nc.tensor.matmul(
                                    psv[j][hh],
                                    lhsT=xh_sb[:, e, kc * 128 : (kc + 1) * 128],
                                    rhs=wv_sb[e][:, hh * 512 : (hh + 1) * 512],
                                    start=(e == 0),
                                    stop=(e == EC - 1),
                                )
                    vstg = stg_pool.tile([128, 4, H], BF16, tag="vstg")
                    for j in range(4):
                        for hh in range(2):
                            hs = slice(hh * 512, (hh + 1) * 512)
                            nc.vector.tensor_add(vstg[:, j, hs], psv[j][hh], bv_bc[:, hs])
                        nc.sync.dma_start(
                            out=vb_in[g * 4 + j, :, :], in_=vstg[:, j, :]
                        )

                # PE order: K0, V0, K1, V1, Q. CC chain (by deadline):
                # K0 after its staging, V (one 2MB gather) after both V
                # groups stage, K1 last.
                k_slice(0)
                v_group(0)
                k_slice(1)

                # wq prefetch into the wk slots (tag reuse -> anti-dep; must
                # be emitted after k_slice(1), the last wk reader)
                wq_sb = [
                    wA_pool.tile([128, H], BF16, tag=f"wA{e}", name=f"wq{e}")
                    for e in range(EC)
                ]
                for e in range(EC):
                    nc.sync.dma_start(out=wq_sb[e], in_=wqT[e * 128 : (e + 1) * 128, :])
                xq_sb = xq_pool.tile([128, EC, NT * 128], BF16, tag="xq")
                for e in range(EC):
                    nc.sync.dma_start(
                        out=xq_sb[:, e, :], in_=xq[e * 128 : (e + 1) * 128, :]
                    )

                v_group(1)
                nc.gpsimd.collective_compute(
                    "AllGather",
                    mybir.AluOpType.bypass,
                    replica_groups=GROUPS,
                    ins=[vb_in[:, :, :].opt()],
                    outs=[vb_out[:, :, :, :].opt()],
                )
                for r in range(2):
                    for j in range(8):
                        nc.scalar.dma_start(
                            out=v_sb[r * 8 + j], in_=vb_out[r, j, :, :]
                        )

                # ---- phase Q: project my 8 q-tiles ------------------------
                for qs in range(2):  # 512-wide query column groups
                    ps = [
                        pps_pool.tile([128, 512], F32, tag="pps", name=f"psq{hc}")
                        for hc in range(HC)
                    ]
                    for e in range(EC):
                        for hc in range(HC):
                            nc.tensor.matmul(
                                ps[hc],
                                lhsT=wq_sb[e][:, hc * 128 : (hc + 1) * 128],
                                rhs=xq_sb[:, e, qs * 512 : (qs + 1) * 512],
                                start=(e == 0),
                                stop=(e == EC - 1),
                            )
                    for hc in range(HC):
                        nc.vector.tensor_scalar_add(
                            qt_all[:, 2 * qs : 2 * qs + 2, hc, :],
                            ps[hc][:, :].rearrange("p (c q) -> p c q", c=2),
                            bq_t[:, hc : hc + 1],
                        )

            # ---- phase 2: attention (scores transposed: [k, q]) -----------
            with (
                tc.tile_pool(name="mskp", bufs=2) as msk_pool,
                tc.tile_pool(name="ptp", bufs=4) as pt_pool,
                tc.tile_pool(name="outp", bufs=2) as out_pool,
                tc.tile_pool(name="stat", bufs=4) as stat_pool,
                tc.tile_pool(name="sps", bufs=3, space="PSUM") as sps_pool,
                tc.tile_pool(name="ops", bufs=1, space="PSUM") as ops_pool,
                tc.tile_pool(name="seps", bufs=1, space="PSUM") as se_pool,
            ):
                # Scores (need only kt) are decoupled from AV (needs v_sb):
                # the exp'd probabilities buffer in SBUF so score work for
                # later classes fills the PE while the V collectives land.
                pts_store = {}

                def emit_scores(cls):
                    nkc = 4 * (cls + 1)
                    msk = msk_pool.tile([128, 4, 256], BF16, tag="msk", name="msk")
                    nc.sync.dma_start(out=msk, in_=masks[cls, :, :, :])
                    for kc in range(nkc):
                        sp = sps_pool.tile([128, 256], F32, tag="sp", name="sp")
                        for hc in range(HC):
                            nc.tensor.matmul(
                                sp,
                                lhsT=kt[hc][:, kc * 128 : (kc + 1) * 128],
                                rhs=qt_all[:, cls, hc, :],
                                start=(hc == 0),
                                stop=(hc == HC - 1),
                            )
                        pt = pt_pool.tile(
                            [128, 256], BF16, tag="pt", bufs=44, name="pt"
                        )
                        nc.scalar.activation(pt, sp, mybir.ActivationFunctionType.Exp)
                        if kc >= 4 * cls:
                            nc.vector.tensor_mul(pt, pt, msk[:, kc - 4 * cls, :])
                        pts_store[(cls, kc)] = pt

                def emit_av(cls):
                    nkc = 4 * (cls + 1)
                    po = [
                        [
                            ops_pool.tile(
                                [128, 512], F32, tag=f"po{t2}{hh}", name=f"po{t2}{hh}"
                            )
                            for hh in range(2)
                        ]
                        for t2 in range(2)
                    ]
                    # one bank for both sums: t2=0's start=True clears the
                    # bank; t2=1's first matmul (start=False) then lands on
                    # has_written=0 elements, i.e. a plain write.
                    se2 = se_pool.tile([128, 2], F32, tag="se", name="se2")
                    se = [se2[:, t2 : t2 + 1] for t2 in range(2)]
                    for kc in range(nkc):
                        pt = pts_store.pop((cls, kc))
                        for t2 in range(2):
                            pts = pt[:, t2 * 128 : (t2 + 1) * 128]
                            for hh in range(2):
                                nc.tensor.matmul(
                                    po[t2][hh],
                                    lhsT=pts,
                                    rhs=v_sb[kc][:, hh * 512 : (hh + 1) * 512],
                                    start=(kc == 0),
                                    stop=(kc == nkc - 1),
                                )
                            nc.tensor.matmul(
                                se[t2],
                                lhsT=pts,
                                rhs=ones,
                                start=(kc == 0 and t2 == 0),
                                stop=(kc == nkc - 1),
                            )
                    for t2 in range(2):
                        rl = stat_pool.tile([128, 1], F32, tag="rl", name="rl")
                        nc.vector.reciprocal(rl, se[t2])
                        ot = out_pool.tile([128, H], F32, tag="ot", name="ot")
                        for hh in range(2):
                            nc.vector.tensor_scalar_mul(
                                ot[:, hh * 512 : (hh + 1) * 512], po[t2][hh], rl
                            )
                        nc.sync.dma_start(out=out[2 * cls + t2, :, :], in_=ot)

                emit_scores(0)
                emit_scores(2)
                emit_av(0)
                emit_av(2)
                emit_scores(1)
                emit_av(1)
                emit_scores(3)
                emit_av(3)

    return nc


def kernel(inputs, Wq, bq, Wk, bk, Wv, bv):
    global LAST_RESULT
    inputs = np.ascontiguousarray(inputs, dtype=np.float32)
    scale = 1.0 / np.sqrt(np.float32(E))

    wqT = np.ascontiguousarray(Wq.T.astype(np.float32) * scale).astype(BF)
    wkT = np.ascontiguousarray(Wk.T.astype(np.float32)).astype(BF)
    wvT = np.ascontiguousarray(Wv.T.astype(np.float32)).astype(BF)
    bqs = (bq.astype(np.float32) * scale).copy()
    bk = np.ascontiguousarray(bk, dtype=np.float32)
    bv = np.ascontiguousarray(bv, dtype=np.float32)

    xTs = [np.ascontiguousarray(inputs[b].T).astype(BF) for b in range(B)]

    in_maps = []
    for c in range(NCORES):
        b = c // 2
        half = c % 2
        xT = xTs[b]
        xh = np.ascontiguousarray(xT[:, half * SH : (half + 1) * SH])
        cols = []
        mask = np.empty((4, 128, 4, 256), dtype=BF)
        karange = np.arange(128)[:, None]
        qarange = np.arange(128)[None, :]
        for t in range(NT):
            r = _global_tile(c, t)
            cols.append(xT[:, r * 128 : (r + 1) * 128])
        for cls in range(4):
            for j in range(4):
                kglob = (4 * cls + j) * 128 + karange
                for t2 in range(2):
                    r = _global_tile(c, 2 * cls + t2)
                    qglob = r * 128 + qarange
                    mask[cls, :, j, t2 * 128 : (t2 + 1) * 128] = (
                        kglob <= qglob
                    ).astype(BF)
        xq = np.ascontiguousarray(np.concatenate(cols, axis=1))
        in_maps.append(
            {
                "xh": xh,
                "xq": xq,
                "wqT": wqT,
                "wkT": wkT,
                "wvT": wvT,
                "bqs": bqs,
                "bk": bk,
                "bv": bv,
                "masks": mask,
            }
        )

    nc = _build_program()
    res = None
    last_err = None
    for attempt in range(3):
        try:
            res = run_bass_kernel_spmd(nc, in_maps, list(range(NCORES)))
            break
        except Exception as e:  # transient NRT device wedge; retry
            last_err = e
            import time as _time

            _time.sleep(2.0)
    if res is None:
        raise last_err
    LAST_RESULT = res

    out = np.empty((B, S, H), dtype=np.float32)
    for c in range(NCORES):
        b = c // 2
        o = res.results[c]["out"]  # [NT, 128, H]
        for t in range(NT):
            r = _global_tile(c, t)
            out[b, r * 128 : (r + 1) * 128, :] = o[t]
    return out


# revision 28
# speedup vs baseline: 1.0542x; 1.0542x over previous
import os
import sys

sys.path.insert(0, "/opt/trn_rl_repo")

import numpy as np
import ml_dtypes
import bass_rust
from concourse import bass, mybir
from concourse.tile import TileContext
from concourse.vector_clock import ScopedClock
from concourse.bass_utils import run_bass_kernel_spmd

B, S, E, H = 4, 2048, 1024, 1024
NCORES = 8
NT = 8  # q-tiles per core, 128 rows each
SH = S // 2  # key half per core
EC = E // 128  # contraction chunks
HC = H // 128  # h chunks
F32 = mybir.dt.float32
BF16 = mybir.dt.bfloat16
BF = ml_dtypes.bfloat16

# Results of the last run_bass_kernel_spmd call (for test harness inspection).
LAST_RESULT = None


def _global_tile(core: int, t: int) -> int:
    """Local q-tile t on core -> global 128-row tile index r in [0, 16).

    Tiles are grouped in four causal classes (nkc = 4*(t//2+1) key-chunks of
    128); each core takes two tiles per class so instruction streams are
    identical across cores.
    """
    half = core % 2
    return 4 * (t // 2) + 2 * half + (t % 2)


class PatchedTileContext(TileContext):
    """TileContext whose tail drain carries at most one sem wait.

    The walrus codegen in this container rejects a Drain with more than one
    sync wait ("Too many sync wait commands"); split the global-clock waits
    across a chain of drains on the same engine instead.
    """

    def _drain_and_barrier(self, tick_clock, wait_clock):
        drain_inst = self.nc.sync.drain()
        wait_clock.add_sem_waits(
            drain_inst.ins, ScopedClock({None: tick_clock.global_clock})
        )
        mi = drain_inst.ins
        waits = list(mi.sync_info.on_wait)
        ups = list(mi.sync_info.on_update)
        if len(waits) > 1:
            mi.sync_info = bass_rust.SyncInfo(on_wait=waits[:1], on_update=[])
            for i, w in enumerate(waits[1:]):
                d2 = self.nc.sync.drain()
                last = i == len(waits) - 2
                d2.ins.sync_info = bass_rust.SyncInfo(
                    on_wait=[w], on_update=ups if last else []
                )
        self.nc.all_engine_barrier()
        assert self.sems is not None
        popped = self.nc._tile_sem_poison_stack.pop()
        assert popped is self._sem_poison
        self.nc.clear_and_free_semaphores(list(self.sems.allocated().values()))
        self.nc.all_engine_barrier()


def _split_multi_waits(json_bytes):
    """Rewrite BIR so no instruction carries more than one sync wait.

    The walrus build in this container rejects instructions with multiple
    sync waits ("Too many sync wait commands"). Engines execute in order, so
    hoisting the extra waits onto NoOp instructions inserted immediately
    before the original instruction is semantically equivalent.
    """
    import json as _json

    d = _json.loads(json_bytes)
    ctr = 0
    for f in d.get("functions", []):
        for blk in f.get("blocks", []):
            insts = blk.get("instructions", [])
            out = []
            for inst in insts:
                si = inst.get("sync_info") or {}
                ow = si.get("on_wait") or []
                if len(ow) > 1:
                    for w in ow[:-1]:
                        out.append(
                            {
                                "debug": inst.get("debug", 0),
                                "engine": inst["engine"],
                                "ins": [],
                                "name": f"wsplit_{ctr}",
                                "opcode": "NoOp",
                                "outs": [],
                                "sync_info": {"on_update": [], "on_wait": [w]},
                            }
                        )
                        ctr += 1
                    si = dict(si)
                    si["on_wait"] = [ow[-1]]
                    inst = dict(inst)
                    inst["sync_info"] = si
                out.append(inst)
            blk["instructions"] = out
    return _json.dumps(d).encode()


def _build_program():
    nc = bass.Bass("TRN2", target_bir_lowering=False, debug=False, num_devices=NCORES)
    orig_to_json_bytes = nc.to_json_bytes
    nc.to_json_bytes = lambda: _split_multi_waits(orig_to_json_bytes())

    xh = nc.dram_tensor("xh", [E, SH], BF16, kind="ExternalInput")
    xq = nc.dram_tensor("xq", [E, NT * 128], BF16, kind="ExternalInput")
    wqT = nc.dram_tensor("wqT", [E, H], BF16, kind="ExternalInput")
    wkT = nc.dram_tensor("wkT", [E, H], BF16, kind="ExternalInput")
    wvT = nc.dram_tensor("wvT", [E, H], BF16, kind="ExternalInput")
    bqs = nc.dram_tensor("bqs", [H], F32, kind="ExternalInput")
    bk = nc.dram_tensor("bk", [H], F32, kind="ExternalInput")
    bv = nc.dram_tensor("bv", [H], F32, kind="ExternalInput")
    masks = nc.dram_tensor("masks", [4, 128, 4, 256], BF16, kind="ExternalInput")
    out = nc.dram_tensor("out", [NT, 128, H], F32, kind="ExternalOutput")

    GROUPS = [[0, 1], [2, 3], [4, 5], [6, 7]]

    with PatchedTileContext(nc) as tc:
        with (
            tc.tile_pool(name="const", bufs=1) as const_pool,
            tc.tile_pool(name="ktp", bufs=1) as kt_pool,
            tc.tile_pool(name="vp", bufs=1) as v_pool,
            tc.tile_pool(name="qtp", bufs=1) as qt_pool,
            tc.tile_pool(name="dram", bufs=1, space="DRAM") as dram_pool,
        ):
            cst = const_pool.tile([128, H + 2 * HC], F32, tag="cst")
            bv_bc = cst[:, 0:H]
            bq_t = cst[:, H : H + HC]
            bk_t = cst[:, H + HC : H + 2 * HC]
            ones = const_pool.tile([128, 1], BF16, tag="ones")
            nc.gpsimd.dma_start(out=bq_t, in_=bqs[:].rearrange("(c p) -> p c", p=128))
            nc.gpsimd.dma_start(out=bk_t, in_=bk[:].rearrange("(c p) -> p c", p=128))
            nc.gpsimd.dma_start(out=bv_bc, in_=bv[:].partition_broadcast(128))
            nc.gpsimd.memset(ones, 1.0)

            kt = [
                kt_pool.tile([128, S], BF16, tag=f"kt{c}", name=f"kt{c}")
                for c in range(HC)
            ]
            v_sb = [
                v_pool.tile([128, H], BF16, tag=f"v{k}", name=f"v{k}")
                for k in range(16)
            ]
            qt_all = qt_pool.tile([128, 4, HC, 256], BF16, tag="qt")

            kb_in = [
                dram_pool.tile([HC, 128, 512], BF16, tag=f"kbi{s}", name=f"kbi{s}")
                for s in range(2)
            ]
            kb_out = [
                dram_pool.tile([2, HC, 128, 512], BF16, tag=f"kbo{s}", name=f"kbo{s}")
                for s in range(2)
            ]
            vb_in = [
                dram_pool.tile([4, 128, H], BF16, tag=f"vbi{g}", name=f"vbi{g}")
                for g in range(2)
            ]
            vb_out = [
                dram_pool.tile([2, 4, 128, H], BF16, tag=f"vbo{g}", name=f"vbo{g}")
                for g in range(2)
            ]

            with (
                tc.tile_pool(name="xhp", bufs=1) as xh_pool,
                tc.tile_pool(name="wA", bufs=1) as wA_pool,
                tc.tile_pool(name="wB", bufs=1) as wB_pool,
                tc.tile_pool(name="xqp", bufs=1) as xq_pool,
                tc.tile_pool(name="stg", bufs=2) as stg_pool,
                tc.tile_pool(name="pps", bufs=8, space="PSUM") as pps_pool,
            ):
                xh_sb = xh_pool.tile([128, EC, SH], BF16, tag="xh")
                wk_sb = [
                    wA_pool.tile([128, H], BF16, tag=f"wA{e}", name=f"wk{e}")
                    for e in range(EC)
                ]
                wv_sb = [
                    wB_pool.tile([128, H], BF16, tag=f"wB{e}", name=f"wv{e}")
                    for e in range(EC)
                ]
                # interleave x chunks with wk chunks so the e=0 matmul can
                # issue after the first two transfers; wv prefetches behind.
                # e=0 loads only the first 512 key-columns up front so the
                # first matmul's data lands as early as possible.
                nc.sync.dma_start(out=wk_sb[0], in_=wkT[0:128, :])
                nc.sync.dma_start(out=xh_sb[:, 0, 0:512], in_=xh[0:128, 0:512])
                for e in range(1, EC):
                    nc.sync.dma_start(
                        out=wk_sb[e], in_=wkT[e * 128 : (e + 1) * 128, :]
                    )
                    nc.sync.dma_start(
                        out=xh_sb[:, e, 0:512], in_=xh[e * 128 : (e + 1) * 128, 0:512]
                    )
                for e in range(EC):
                    nc.sync.dma_start(
                        out=xh_sb[:, e, 512:SH], in_=xh[e * 128 : (e + 1) * 128, 512:SH]
                    )
                for e in range(EC):
                    nc.sync.dma_start(out=wv_sb[e], in_=wvT[e * 128 : (e + 1) * 128, :])

                # ---- phases K/V interleaved: each 512-key piece stages and
                # kicks its pairwise AllGather as early as possible so the
                # CC chain (K0, V0, K1, V1) hides under later projections.
                kth = stg_pool.tile([128, HC, SH], BF16, tag="kth", bufs=1)

                def k_slice(sl):
                    psk = [
                        pps_pool.tile([128, 512], F32, tag="pps", name=f"psk{hc}")
                        for hc in range(HC)
                    ]
                    for e in range(EC):
                        for hc in range(HC):
                            nc.tensor.matmul(
                                psk[hc],
                                lhsT=wk_sb[e][:, hc * 128 : (hc + 1) * 128],
                                rhs=xh_sb[:, e, sl * 512 : (sl + 1) * 512],
                                start=(e == 0),
                                stop=(e == EC - 1),
                            )
                    for hc in range(HC):
                        nc.vector.tensor_scalar_add(
                            kth[:, hc, sl * 512 : (sl + 1) * 512],
                            psk[hc],
                            bk_t[:, hc : hc + 1],
                        )
                        nc.sync.dma_start(
                            out=kb_in[sl][hc, :, :],
                            in_=kth[:, hc, sl * 512 : (sl + 1) * 512],
                        )
                    nc.gpsimd.collective_compute(
                        "AllGather",
                        mybir.AluOpType.bypass,
                        replica_groups=GROUPS,
                        ins=[kb_in[sl][:, :, :].opt()],
                        outs=[kb_out[sl][:, :, :, :].opt()],
                    )
                    # readbacks trigger from scalar so they don't head-block
                    # the sync queue on the CC-done semaphore
                    for r in range(2):
                        for hc in range(HC):
                            nc.scalar.dma_start(
                                out=kt[hc][
                                    :, r * SH + sl * 512 : r * SH + (sl + 1) * 512
                                ],
                                in_=kb_out[sl][r, hc, :, :],
                            )

                def v_group(g):
                    psv = [
                        [
                            pps_pool.tile(
                                [128, 512], F32, tag="pps", name=f"psv{j}_{hh}"
                            )
                            for hh in range(2)
                        ]
                        for j in range(4)
                    ]
                    for e in range(EC):
                        for j in range(4):
                            kc = g * 4 + j
                            for hh in range(2):
                                nc.tensor.matmul(
                                    psv[j][hh],
                                    lhsT=xh_sb[:, e, kc * 128 : (kc + 1) * 128],
                                    rhs=wv_sb[e][:, hh * 512 : (hh + 1) * 512],
                                    start=(e == 0),
                                    stop=(e == EC - 1),
                                )
                    vstg = stg_pool.tile([128, 4, H], BF16, tag="vstg")
                    for j in range(4):
                        for hh in range(2):
                            hs = slice(hh * 512, (hh + 1) * 512)
                            nc.vector.tensor_add(vstg[:, j, hs], psv[j][hh], bv_bc[:, hs])
                        nc.sync.dma_start(out=vb_in[g][j, :, :], in_=vstg[:, j, :])
                    nc.gpsimd.collective_compute(
                        "AllGather",
                        mybir.AluOpType.bypass,
                        replica_groups=GROUPS,
                        ins=[vb_in[g][:, :, :].opt()],
                        outs=[vb_out[g][:, :, :, :].opt()],
                    )
                    for r in range(2):
                        for j in range(4):
                            nc.scalar.dma_start(
                                out=v_sb[r * 8 + g * 4 + j],
                                in_=vb_out[g][r, j, :, :],
                            )

                # PE order: K0, V0, K1, V1, Q; CC chain K0, V0, K1, V1.
                k_slice(0)
                v_group(0)
                k_slice(1)

                # wq prefetch into the wk slots (tag reuse -> anti-dep; must
                # be emitted after k_slice(1), the last wk reader)
                wq_sb = [
                    wA_pool.tile([128, H], BF16, tag=f"wA{e}", name=f"wq{e}")
                    for e in range(EC)
                ]
                for e in range(EC):
                    nc.sync.dma_start(out=wq_sb[e], in_=wqT[e * 128 : (e + 1) * 128, :])
                xq_sb = xq_pool.tile([128, EC, NT * 128], BF16, tag="xq")
                for e in range(EC):
                    nc.sync.dma_start(
                        out=xq_sb[:, e, :], in_=xq[e * 128 : (e + 1) * 128, :]
                    )

                v_group(1)

                # ---- phase Q: project my 8 q-tiles ------------------------
                for qs in range(2):  # 512-wide query column groups
                    ps = [
                        pps_pool.tile([128, 512], F32, tag="pps", name=f"psq{hc}")
                        for hc in range(HC)
                    ]
                    for e in range(EC):
                        for hc in range(HC):
                            nc.tensor.matmul(
                                ps[hc],
                                lhsT=wq_sb[e][:, hc * 128 : (hc + 1) * 128],
                                rhs=xq_sb[:, e, qs * 512 : (qs + 1) * 512],
                                start=(e == 0),
                                stop=(e == EC - 1),
                            )
                    for hc in range(HC):
                        nc.vector.tensor_scalar_add(
                            qt_all[:, 2 * qs : 2 * qs + 2, hc, :],
                            ps[hc][:, :].rearrange("p (c q) -> p c q", c=2),
                            bq_t[:, hc : hc + 1],
                        )

            # ---- phase 2: attention (scores transposed: [k, q]) -----------
            with (
                tc.tile_pool(name="mskp", bufs=2) as msk_pool,
                tc.tile_pool(name="ptp", bufs=4) as pt_pool,
                tc.tile_pool(name="outp", bufs=2) as out_pool,
                tc.tile_pool(name="stat", bufs=4) as stat_pool,
                tc.tile_pool(name="sps", bufs=3, space="PSUM") as sps_pool,
                tc.tile_pool(name="ops", bufs=1, space="PSUM") as ops_pool,
                tc.tile_pool(name="seps", bufs=1, space="PSUM") as se_pool,
            ):
                # Scores (need only kt) are decoupled from AV (needs v_sb):
                # the exp'd probabilities buffer in SBUF so score work for
                # later classes fills the PE while the V collectives land.
                pts_store = {}

                def emit_scores(cls):
                    nkc = 4 * (cls + 1)
                    msk = msk_pool.tile([128, 4, 256], BF16, tag="msk", name="msk")
                    nc.sync.dma_start(out=msk, in_=masks[cls, :, :, :])
                    for kc in range(nkc):
                        sp = sps_pool.tile([128, 256], F32, tag="sp", name="sp")
                        for hc in range(HC):
                            nc.tensor.matmul(
                                sp,
                                lhsT=kt[hc][:, kc * 128 : (kc + 1) * 128],
                                rhs=qt_all[:, cls, hc, :],
                                start=(hc == 0),
                                stop=(hc == HC - 1),
                            )
                        pt = pt_pool.tile(
                            [128, 256], BF16, tag="pt", bufs=44, name="pt"
                        )
                        nc.scalar.activation(pt, sp, mybir.ActivationFunctionType.Exp)
                        if kc >= 4 * cls:
                            nc.vector.tensor_mul(pt, pt, msk[:, kc - 4 * cls, :])
                        pts_store[(cls, kc)] = pt

                def emit_av(cls):
                    nkc = 4 * (cls + 1)
                    po = [
                        [
                            ops_pool.tile(
                                [128, 512], F32, tag=f"po{t2}{hh}", name=f"po{t2}{hh}"
                            )
                            for hh in range(2)
                        ]
                        for t2 in range(2)
                    ]
                    # one bank for both sums: t2=0's start=True clears the
                    # bank; t2=1's first matmul (start=False) then lands on
                    # has_written=0 elements, i.e. a plain write.
                    se2 = se_pool.tile([128, 2], F32, tag="se", name="se2")
                    se = [se2[:, t2 : t2 + 1] for t2 in range(2)]
                    for kc in range(nkc):
                        pt = pts_store.pop((cls, kc))
                        for t2 in range(2):
                            pts = pt[:, t2 * 128 : (t2 + 1) * 128]
                            for hh in range(2):
                                nc.tensor.matmul(
                                    po[t2][hh],
                                    lhsT=pts,
                                    rhs=v_sb[kc][:, hh * 512 : (hh + 1) * 512],
                                    start=(kc == 0),
                                    stop=(kc == nkc - 1),
                                )
                            nc.tensor.matmul(
                                se[t2],
                                lhsT=pts,
                                rhs=ones,
                                start=(kc == 0 and t2 == 0),
                                stop=(kc == nkc - 1),
                            )
                    for t2 in range(2):
                        rl = stat_pool.tile([128, 1], F32, tag="rl", name="rl")
                        nc.vector.reciprocal(rl, se[t2])
                        ot = out_pool.tile([128, H], F32, tag="ot", name="ot")
                        for hh in range(2):
                            nc.vector.tensor_scalar_mul(
                                ot[:, hh * 512 : (hh + 1) * 512], po[t2][hh], rl
                            )
                        nc.sync.dma_start(out=out[2 * cls + t2, :, :], in_=ot)

                emit_scores(0)
                emit_scores(2)
                emit_av(0)
                emit_scores(1)
                emit_scores(3)
                emit_av(2)
                emit_av(1)
                emit_av(3)

    return nc


def kernel(inputs, Wq, bq, Wk, bk, Wv, bv):
    global LAST_RESULT
    inputs = np.ascontiguousarray(inputs, dtype=np.float32)
    scale = 1.0 / np.sqrt(np.float32(E))

    wqT = np.ascontiguousarray(Wq.T.astype(np.float32) * scale).astype(BF)
    wkT = np.ascontiguousarray(Wk.T.astype(np.float32)).astype(BF)
    wvT = np.ascontiguousarray(Wv.T.astype(np.float32)).astype(BF)
    bqs = (bq.astype(np.float32) * scale).copy()
    bk = np.ascontiguousarray(bk, dtype=np.float32)
    bv = np.ascontiguousarray(bv, dtype=np.float32)

    xTs = [np.ascontiguousarray(inputs[b].T).astype(BF) for b in range(B)]

    in_maps = []
    for c in range(NCORES):
        b = c // 2
        half = c % 2
        xT = xTs[b]
        xh = np.ascontiguousarray(xT[:, half * SH : (half + 1) * SH])
        cols = []
        mask = np.empty((4, 128, 4, 256), dtype=BF)
        karange = np.arange(128)[:, None]
        qarange = np.arange(128)[None, :]
        for t in range(NT):
            r = _global_tile(c, t)
            cols.append(xT[:, r * 128 : (r + 1) * 128])
        for cls in range(4):
            for j in range(4):
                kglob = (4 * cls + j) * 128 + karange
                for t2 in range(2):
                    r = _global_tile(c, 2 * cls + t2)
                    qglob = r * 128 + qarange
                    mask[cls, :, j, t2 * 128 : (t2 + 1) * 128] = (
                        kglob <= qglob
                    ).astype(BF)
        xq = np.ascontiguousarray(np.concatenate(cols, axis=1))
        in_maps.append(
            {
                "xh": xh,
                "xq": xq,
                "wqT": wqT,
                "wkT": wkT,
                "wvT": wvT,
                "bqs": bqs,
                "bk": bk,
                "bv": bv,
                "masks": mask,
            }
        )

    nc = _build_program()
    res = None
    last_err = None
    for attempt in range(3):
        try:
            res = run_bass_kernel_spmd(nc, in_maps, list(range(NCORES)))
            break
        except Exception as e:  # transient NRT device wedge; retry
            last_err = e
            import time as _time

            _time.sleep(2.0)
    if res is None:
        raise last_err
    LAST_RESULT = res

    out = np.empty((B, S, H), dtype=np.float32)
    for c in range(NCORES):
        b = c // 2
        o = res.results[c]["out"]  # [NT, 128, H]
        for t in range(NT):
            r = _global_tile(c, t)
            out[b, r * 128 : (r + 1) * 128, :] = o[t]
    return out


# revision 29
# speedup vs baseline: 1.0782x; 1.0228x over previous
import os
import sys

sys.path.insert(0, "/opt/trn_rl_repo")

import numpy as np
import ml_dtypes
import bass_rust
from concourse import bass, mybir
from concourse.tile import TileContext
from concourse.vector_clock import ScopedClock
from concourse.bass_utils import run_bass_kernel_spmd

B, S, E, H = 4, 2048, 1024, 1024
NCORES = 8
NT = 8  # q-tiles per core, 128 rows each
SH = S // 2  # key half per core
EC = E // 128  # contraction chunks
HC = H // 128  # h chunks
F32 = mybir.dt.float32
BF16 = mybir.dt.bfloat16
BF = ml_dtypes.bfloat16

# Results of the last run_bass_kernel_spmd call (for test harness inspection).
LAST_RESULT = None


def _global_tile(core: int, t: int) -> int:
    """Local q-tile t on core -> global 128-row tile index r in [0, 16).

    Tiles are grouped in four causal classes (nkc = 4*(t//2+1) key-chunks of
    128); each core takes two tiles per class so instruction streams are
    identical across cores.
    """
    half = core % 2
    return 4 * (t // 2) + 2 * half + (t % 2)


class PatchedTileContext(TileContext):
    """TileContext whose tail drain carries at most one sem wait.

    The walrus codegen in this container rejects a Drain with more than one
    sync wait ("Too many sync wait commands"); split the global-clock waits
    across a chain of drains on the same engine instead.
    """

    def _drain_and_barrier(self, tick_clock, wait_clock):
        drain_inst = self.nc.sync.drain()
        wait_clock.add_sem_waits(
            drain_inst.ins, ScopedClock({None: tick_clock.global_clock})
        )
        mi = drain_inst.ins
        waits = list(mi.sync_info.on_wait)
        ups = list(mi.sync_info.on_update)
        if len(waits) > 1:
            mi.sync_info = bass_rust.SyncInfo(on_wait=waits[:1], on_update=[])
            for i, w in enumerate(waits[1:]):
                d2 = self.nc.sync.drain()
                last = i == len(waits) - 2
                d2.ins.sync_info = bass_rust.SyncInfo(
                    on_wait=[w], on_update=ups if last else []
                )
        self.nc.all_engine_barrier()
        assert self.sems is not None
        popped = self.nc._tile_sem_poison_stack.pop()
        assert popped is self._sem_poison
        self.nc.clear_and_free_semaphores(list(self.sems.allocated().values()))
        self.nc.all_engine_barrier()


def _split_multi_waits(json_bytes):
    """Rewrite BIR so no instruction carries more than one sync wait.

    The walrus build in this container rejects instructions with multiple
    sync waits ("Too many sync wait commands"). Engines execute in order, so
    hoisting the extra waits onto NoOp instructions inserted immediately
    before the original instruction is semantically equivalent.
    """
    import json as _json

    d = _json.loads(json_bytes)
    ctr = 0
    for f in d.get("functions", []):
        for blk in f.get("blocks", []):
            insts = blk.get("instructions", [])
            out = []
            for inst in insts:
                si = inst.get("sync_info") or {}
                ow = si.get("on_wait") or []
                if len(ow) > 1:
                    for w in ow[:-1]:
                        out.append(
                            {
                                "debug": inst.get("debug", 0),
                                "engine": inst["engine"],
                                "ins": [],
                                "name": f"wsplit_{ctr}",
                                "opcode": "NoOp",
                                "outs": [],
                                "sync_info": {"on_update": [], "on_wait": [w]},
                            }
                        )
                        ctr += 1
                    si = dict(si)
                    si["on_wait"] = [ow[-1]]
                    inst = dict(inst)
                    inst["sync_info"] = si
                out.append(inst)
            blk["instructions"] = out
    return _json.dumps(d).encode()


def _build_program():
    nc = bass.Bass("TRN2", target_bir_lowering=False, debug=False, num_devices=NCORES)
    orig_to_json_bytes = nc.to_json_bytes
    nc.to_json_bytes = lambda: _split_multi_waits(orig_to_json_bytes())

    xh = nc.dram_tensor("xh", [E, SH], BF16, kind="ExternalInput")
    xq = nc.dram_tensor("xq", [E, NT * 128], BF16, kind="ExternalInput")
    wqT = nc.dram_tensor("wqT", [E, H], BF16, kind="ExternalInput")
    wkT = nc.dram_tensor("wkT", [E, H], BF16, kind="ExternalInput")
    wvT = nc.dram_tensor("wvT", [E, H], BF16, kind="ExternalInput")
    bqs = nc.dram_tensor("bqs", [H], F32, kind="ExternalInput")
    bk = nc.dram_tensor("bk", [H], F32, kind="ExternalInput")
    bv = nc.dram_tensor("bv", [H], F32, kind="ExternalInput")
    masks = nc.dram_tensor("masks", [4, 128, 4, 256], BF16, kind="ExternalInput")
    out = nc.dram_tensor("out", [NT, 128, H], F32, kind="ExternalOutput")

    GROUPS = [[0, 1], [2, 3], [4, 5], [6, 7]]

    with PatchedTileContext(nc) as tc:
        with (
            tc.tile_pool(name="const", bufs=1) as const_pool,
            tc.tile_pool(name="ktp", bufs=1) as kt_pool,
            tc.tile_pool(name="vp", bufs=1) as v_pool,
            tc.tile_pool(name="qtp", bufs=1) as qt_pool,
            tc.tile_pool(name="dram", bufs=1, space="DRAM") as dram_pool,
        ):
            cst = const_pool.tile([128, H + 2 * HC], F32, tag="cst")
            bv_bc = cst[:, 0:H]
            bq_t = cst[:, H : H + HC]
            bk_t = cst[:, H + HC : H + 2 * HC]
            ones = const_pool.tile([128, 1], BF16, tag="ones")
            nc.gpsimd.dma_start(out=bq_t, in_=bqs[:].rearrange("(c p) -> p c", p=128))
            nc.gpsimd.dma_start(out=bk_t, in_=bk[:].rearrange("(c p) -> p c", p=128))
            nc.gpsimd.dma_start(out=bv_bc, in_=bv[:].partition_broadcast(128))
            nc.gpsimd.memset(ones, 1.0)

            kt = [
                kt_pool.tile([128, S], BF16, tag=f"kt{c}", name=f"kt{c}")
                for c in range(HC)
            ]
            v_sb = [
                v_pool.tile([128, H], BF16, tag=f"v{k}", name=f"v{k}")
                for k in range(16)
            ]
            qt_all = qt_pool.tile([128, 4, HC, 256], BF16, tag="qt")

            kb_in = [
                dram_pool.tile([HC, 128, 512], BF16, tag=f"kbi{s}", name=f"kbi{s}")
                for s in range(2)
            ]
            kb_out = [
                dram_pool.tile([2, HC, 128, 512], BF16, tag=f"kbo{s}", name=f"kbo{s}")
                for s in range(2)
            ]
            vb_in = [
                dram_pool.tile([4, 128, H], BF16, tag=f"vbi{g}", name=f"vbi{g}")
                for g in range(2)
            ]
            vb_out = [
                dram_pool.tile([2, 4, 128, H], BF16, tag=f"vbo{g}", name=f"vbo{g}")
                for g in range(2)
            ]

            with (
                tc.tile_pool(name="xhp", bufs=1) as xh_pool,
                tc.tile_pool(name="wA", bufs=1) as wA_pool,
                tc.tile_pool(name="wB", bufs=1) as wB_pool,
                tc.tile_pool(name="xqp", bufs=1) as xq_pool,
                tc.tile_pool(name="stg", bufs=2) as stg_pool,
                tc.tile_pool(name="pps", bufs=8, space="PSUM") as pps_pool,
            ):
                xh_sb = xh_pool.tile([128, EC, SH], BF16, tag="xh")
                wk_sb = [
                    wA_pool.tile([128, H], BF16, tag=f"wA{e}", name=f"wk{e}")
                    for e in range(EC)
                ]
                wv_sb = [
                    wB_pool.tile([128, H], BF16, tag=f"wB{e}", name=f"wv{e}")
                    for e in range(EC)
                ]
                # interleave x chunks with wk chunks so the e=0 matmul can
                # issue after the first two transfers; wv prefetches behind.
                # e=0 loads only the first 512 key-columns up front so the
                # first matmul's data lands as early as possible.
                nc.sync.dma_start(out=wk_sb[0], in_=wkT[0:128, :])
                nc.sync.dma_start(out=xh_sb[:, 0, 0:512], in_=xh[0:128, 0:512])
                for e in range(1, EC):
                    nc.sync.dma_start(
                        out=wk_sb[e], in_=wkT[e * 128 : (e + 1) * 128, :]
                    )
                    nc.sync.dma_start(
                        out=xh_sb[:, e, 0:512], in_=xh[e * 128 : (e + 1) * 128, 0:512]
                    )
                for e in range(EC):
                    nc.sync.dma_start(
                        out=xh_sb[:, e, 512:SH], in_=xh[e * 128 : (e + 1) * 128, 512:SH]
                    )
                for e in range(EC):
                    nc.sync.dma_start(out=wv_sb[e], in_=wvT[e * 128 : (e + 1) * 128, :])

                # ---- phases K/V interleaved: each 512-key piece stages and
                # kicks its pairwise AllGather as early as possible so the
                # CC chain (K0, V0, K1, V1) hides under later projections.
                kth = stg_pool.tile([128, HC, SH], BF16, tag="kth", bufs=1)

                def k_slice(sl):
                    psk = [
                        pps_pool.tile([128, 512], F32, tag="pps", name=f"psk{hc}")
                        for hc in range(HC)
                    ]
                    for e in range(EC):
                        for hc in range(HC):
                            nc.tensor.matmul(
                                psk[hc],
                                lhsT=wk_sb[e][:, hc * 128 : (hc + 1) * 128],
                                rhs=xh_sb[:, e, sl * 512 : (sl + 1) * 512],
                                start=(e == 0),
                                stop=(e == EC - 1),
                            )
                    for hc in range(HC):
                        nc.vector.tensor_scalar_add(
                            kth[:, hc, sl * 512 : (sl + 1) * 512],
                            psk[hc],
                            bk_t[:, hc : hc + 1],
                        )
                        nc.sync.dma_start(
                            out=kb_in[sl][hc, :, :],
                            in_=kth[:, hc, sl * 512 : (sl + 1) * 512],
                        )
                    nc.gpsimd.collective_compute(
                        "AllGather",
                        mybir.AluOpType.bypass,
                        replica_groups=GROUPS,
                        ins=[kb_in[sl][:, :, :].opt()],
                        outs=[kb_out[sl][:, :, :, :].opt()],
                    )
                    # readbacks trigger from scalar so they don't head-block
                    # the sync queue on the CC-done semaphore
                    for r in range(2):
                        for hc in range(HC):
                            nc.scalar.dma_start(
                                out=kt[hc][
                                    :, r * SH + sl * 512 : r * SH + (sl + 1) * 512
                                ],
                                in_=kb_out[sl][r, hc, :, :],
                            )

                def v_group(g):
                    psv = [
                        [
                            pps_pool.tile(
                                [128, 512], F32, tag="pps", name=f"psv{j}_{hh}"
                            )
                            for hh in range(2)
                        ]
                        for j in range(4)
                    ]
                    for e in range(EC):
                        for j in range(4):
                            kc = g * 4 + j
                            for hh in range(2):
                                nc.tensor.matmul(
                                    psv[j][hh],
                                    lhsT=xh_sb[:, e, kc * 128 : (kc + 1) * 128],
                                    rhs=wv_sb[e][:, hh * 512 : (hh + 1) * 512],
                                    start=(e == 0),
                                    stop=(e == EC - 1),
                                )
                    vstg = stg_pool.tile([128, 4, H], BF16, tag="vstg")
                    for j in range(4):
                        for hh in range(2):
                            hs = slice(hh * 512, (hh + 1) * 512)
                            nc.vector.tensor_add(vstg[:, j, hs], psv[j][hh], bv_bc[:, hs])
                        nc.sync.dma_start(out=vb_in[g][j, :, :], in_=vstg[:, j, :])
                    nc.gpsimd.collective_compute(
                        "AllGather",
                        mybir.AluOpType.bypass,
                        replica_groups=GROUPS,
                        ins=[vb_in[g][:, :, :].opt()],
                        outs=[vb_out[g][:, :, :, :].opt()],
                    )
                    for r in range(2):
                        for j in range(4):
                            nc.scalar.dma_start(
                                out=v_sb[r * 8 + g * 4 + j],
                                in_=vb_out[g][r, j, :, :],
                            )

                # PE order: K0, V0, K1, V1, Q; CC chain K0, V0, K1, V1.
                k_slice(0)
                v_group(0)
                k_slice(1)

                # wq prefetch into the wk slots (tag reuse -> anti-dep; must
                # be emitted after k_slice(1), the last wk reader)
                wq_sb = [
                    wA_pool.tile([128, H], BF16, tag=f"wA{e}", name=f"wq{e}")
                    for e in range(EC)
                ]
                for e in range(EC):
                    nc.sync.dma_start(out=wq_sb[e], in_=wqT[e * 128 : (e + 1) * 128, :])
                xq_sb = xq_pool.tile([128, EC, NT * 128], BF16, tag="xq")
                for e in range(EC):
                    nc.sync.dma_start(
                        out=xq_sb[:, e, :], in_=xq[e * 128 : (e + 1) * 128, :]
                    )

                v_group(1)

                # ---- phase Q: project my 8 q-tiles ------------------------
                for qs in range(2):  # 512-wide query column groups
                    ps = [
                        pps_pool.tile([128, 512], F32, tag="pps", name=f"psq{hc}")
                        for hc in range(HC)
                    ]
                    for e in range(EC):
                        for hc in range(HC):
                            nc.tensor.matmul(
                                ps[hc],
                                lhsT=wq_sb[e][:, hc * 128 : (hc + 1) * 128],
                                rhs=xq_sb[:, e, qs * 512 : (qs + 1) * 512],
                                start=(e == 0),
                                stop=(e == EC - 1),
                            )
                    for hc in range(HC):
                        nc.vector.tensor_scalar_add(
                            qt_all[:, 2 * qs : 2 * qs + 2, hc, :],
                            ps[hc][:, :].rearrange("p (c q) -> p c q", c=2),
                            bq_t[:, hc : hc + 1],
                        )

            # ---- phase 2: attention (scores transposed: [k, q]) -----------
            with (
                tc.tile_pool(name="mskp", bufs=2) as msk_pool,
                tc.tile_pool(name="ptp", bufs=4) as pt_pool,
                tc.tile_pool(name="outp", bufs=2) as out_pool,
                tc.tile_pool(name="stat", bufs=4) as stat_pool,
                tc.tile_pool(name="sps", bufs=3, space="PSUM") as sps_pool,
                tc.tile_pool(name="ops", bufs=1, space="PSUM") as ops_pool,
                tc.tile_pool(name="seps", bufs=1, space="PSUM") as se_pool,
            ):
                # Scores (need only kt) are decoupled from AV (needs v_sb):
                # the exp'd probabilities buffer in SBUF so score work for
                # later classes fills the PE while the V collectives land.
                pts_store = {}

                def emit_scores(cls):
                    nkc = 4 * (cls + 1)
                    msk = msk_pool.tile([128, 4, 256], BF16, tag="msk", name="msk")
                    nc.sync.dma_start(out=msk, in_=masks[cls, :, :, :])
                    for kc in range(nkc):
                        sp = sps_pool.tile([128, 256], F32, tag="sp", name="sp")
                        for hc in range(HC):
                            nc.tensor.matmul(
                                sp,
                                lhsT=kt[hc][:, kc * 128 : (kc + 1) * 128],
                                rhs=qt_all[:, cls, hc, :],
                                start=(hc == 0),
                                stop=(hc == HC - 1),
                            )
                        pt = pt_pool.tile(
                            [128, 256], BF16, tag="pt", bufs=44, name="pt"
                        )
                        nc.scalar.activation(pt, sp, mybir.ActivationFunctionType.Exp)
                        if kc >= 4 * cls:
                            nc.vector.tensor_mul(pt, pt, msk[:, kc - 4 * cls, :])
                        pts_store[(cls, kc)] = pt

                def emit_av(cls):
                    nkc = 4 * (cls + 1)
                    po = [
                        [
                            ops_pool.tile(
                                [128, 512], F32, tag=f"po{t2}{hh}", name=f"po{t2}{hh}"
                            )
                            for hh in range(2)
                        ]
                        for t2 in range(2)
                    ]
                    # one bank for both sums: t2=0's start=True clears the
                    # bank; t2=1's first matmul (start=False) then lands on
                    # has_written=0 elements, i.e. a plain write.
                    se2 = se_pool.tile([128, 2], F32, tag="se", name="se2")
                    se = [se2[:, t2 : t2 + 1] for t2 in range(2)]
                    # PSUM accumulation is order-independent: consume the
                    # chunks carried by the first V collective (0-3, 8-11)
                    # before the late-landing ones (4-7, 12-15)
                    kcs = [k for k in range(nkc) if (k // 4) % 2 == 0] + [
                        k for k in range(nkc) if (k // 4) % 2 == 1
                    ]
                    for i, kc in enumerate(kcs):
                        pt = pts_store.pop((cls, kc))
                        for t2 in range(2):
                            pts = pt[:, t2 * 128 : (t2 + 1) * 128]
                            for hh in range(2):
                                nc.tensor.matmul(
                                    po[t2][hh],
                                    lhsT=pts,
                                    rhs=v_sb[kc][:, hh * 512 : (hh + 1) * 512],
                                    start=(i == 0),
                                    stop=(i == nkc - 1),
                                )
                            nc.tensor.matmul(
                                se[t2],
                                lhsT=pts,
                                rhs=ones,
                                start=(i == 0 and t2 == 0),
                                stop=(i == nkc - 1),
                            )
                    for t2 in range(2):
                        rl = stat_pool.tile([128, 1], F32, tag="rl", name="rl")
                        nc.vector.reciprocal(rl, se[t2])
                        ot = out_pool.tile([128, H], F32, tag="ot", name="ot")
                        for hh in range(2):
                            nc.vector.tensor_scalar_mul(
                                ot[:, hh * 512 : (hh + 1) * 512], po[t2][hh], rl
                            )
                        nc.sync.dma_start(out=out[2 * cls + t2, :, :], in_=ot)

                emit_scores(0)
                emit_scores(2)
                emit_av(0)
                emit_scores(1)
                emit_scores(3)
                emit_av(2)
                emit_av(1)
                emit_av(3)

    return nc


def kernel(inputs, Wq, bq, Wk, bk, Wv, bv):
    global LAST_RESULT
    inputs = np.ascontiguousarray(inputs, dtype=np.float32)
    scale = 1.0 / np.sqrt(np.float32(E))

    wqT = np.ascontiguousarray(Wq.T.astype(np.float32) * scale).astype(BF)
    wkT = np.ascontiguousarray(Wk.T.astype(np.float32)).astype(BF)
    wvT = np.ascontiguousarray(Wv.T.astype(np.float32)).astype(BF)
    bqs = (bq.astype(np.float32) * scale).copy()
    bk = np.ascontiguousarray(bk, dtype=np.float32)
    bv = np.ascontiguousarray(bv, dtype=np.float32)

    xTs = [np.ascontiguousarray(inputs[b].T).astype(BF) for b in range(B)]

    in_maps = []
    for c in range(NCORES):
        b = c // 2
        half = c % 2
        xT = xTs[b]
        xh = np.ascontiguousarray(xT[:, half * SH : (half + 1) * SH])
        cols = []
        mask = np.empty((4, 128, 4, 256), dtype=BF)
        karange = np.arange(128)[:, None]
        qarange = np.arange(128)[None, :]
        for t in range(NT):
            r = _global_tile(c, t)
            cols.append(xT[:, r * 128 : (r + 1) * 128])
        for cls in range(4):
            for j in range(4):
                kglob = (4 * cls + j) * 128 + karange
                for t2 in range(2):
                    r = _global_tile(c, 2 * cls + t2)
                    qglob = r * 128 + qarange
                    mask[cls, :, j, t2 * 128 : (t2 + 1) * 128] = (
                        kglob <= qglob
                    ).astype(BF)
        xq = np.ascontiguousarray(np.concatenate(cols, axis=1))
        in_maps.append(
            {
                "xh": xh,
                "xq": xq,
                "wqT": wqT,
                "wkT": wkT,
                "wvT": wvT,
                "bqs": bqs,
                "bk": bk,
                "bv": bv,
                "masks": mask,
            }
        )

    nc = _build_program()
    res = None
    last_err = None
    for attempt in range(3):
        try:
            res = run_bass_kernel_spmd(nc, in_maps, list(range(NCORES)))
            break
        except Exception as e:  # transient NRT device wedge; retry
            last_err = e
            import time as _time

            _time.sleep(2.0)
    if res is None:
        raise last_err
    LAST_RESULT = res

    out = np.empty((B, S, H), dtype=np.float32)
    for c in range(NCORES):
        b = c // 2
        o = res.results[c]["out"]  # [NT, 128, H]
        for t in range(NT):
            r = _global_tile(c, t)
            out[b, r * 128 : (r + 1) * 128, :] = o[t]
    return out
